# revision 1
# baseline (speedup 1.0000x reference)
"""CNLP (common-neighbor link prediction) kernel for Trainium2, 8 NeuronCores.

Reference computation (per query edge e = (i, j)):
    cn  = adj[i] * adj[j]                      # common-neighbor indicator [N]
    xcn = cn @ x                               # sum of common-neighbor feats
    xij = relu(x[i]*x[j] @ Wa.T + ba) @ Wb.T + bb
    hcn = (relu->relu->lin) 3-layer MLP on xcn
    out = (hcn * beta + xij) @ Wl.T + bl       # [E, 1]

Sharding: edges (E=8192) split 8 x 1024 across cores; adj/x/weights replicated.

Device strategy per core:
  - adj is binary -> uploaded as bf16 (exact), padded N->10240.
  - x uploaded as a [10240, 512] bf16 table of [x_hi | x_lo] (hi/lo split of
    fp32, so bf16 matmuls recover ~16-bit-mantissa accuracy).
  - gpsimd dma_gather(transpose=True) gathers adjacency rows for a batch of
    128 edges directly in TRANSPOSED layout [128 n-part, chunk, edge] via the
    DMA xbar; DVE multiplies the two gathered tiles in place -> cn^T.
  - TensorE accumulates [cn@x_hi | cn@x_lo] over 80 n-chunks into PSUM;
    DVE adds the halves -> xcn fp32; two PE transposes flip each edge batch
    to feature-major xcn^T.
  - All MLPs run in fp32, feature-major ([feat, edge]), biases+ReLU fused
    into ScalarE activations.  Final output [1, 1024] fp32 per core.
"""

import numpy as np
import ml_dtypes

import concourse.bacc as bacc
import concourse.tile as tile
import concourse.mybir as mybir
from concourse.bass_utils import run_bass_kernel_spmd

BF16 = mybir.dt.bfloat16
FP32 = mybir.dt.float32
FP8 = mybir.dt.float8e4
I16 = mybir.dt.int16
AF = mybir.ActivationFunctionType
BF16_NP = ml_dtypes.bfloat16
FP8_NP = ml_dtypes.float8_e4m3

N_CORES = 8
N, E, D, H = 10000, 8192, 256, 256
NPAD = 10240                      # n padded to a multiple of 256
EC = E // N_CORES                 # 1024 edges per core
EB = 128                          # edges per batch (one gather tile)


def build_program(n=N, npad=NPAD, ec=EC, n_halves=2):
    nch = npad // 128              # n chunks of 128
    ch_h = nch // n_halves         # chunks per gather tile
    nb = ec // EB                  # edge batches
    ehw = min(512, ec)             # edge columns per fp32 matmul
    n_eh = ec // ehw

    nc = bacc.Bacc("TRN2", target_bir_lowering=False, debug=False,
                   enable_asserts=False, num_devices=N_CORES)

    adjb = nc.dram_tensor("adjb", [n, npad], FP8, kind="ExternalInput")
    xhl_d = nc.dram_tensor("xhl", [npad, 2 * D], BF16, kind="ExternalInput")
    # adjacency-gather indices (raw node ids) and x-gather indices
    # (positions in the interleave-permuted xhl table)
    idx0_d = nc.dram_tensor("idx0", [128, ec // 16], I16, kind="ExternalInput")
    idx1_d = nc.dram_tensor("idx1", [128, ec // 16], I16, kind="ExternalInput")
    idx0x_d = nc.dram_tensor("idx0x", [128, ec // 16], I16, kind="ExternalInput")
    idx1x_d = nc.dram_tensor("idx1x", [128, ec // 16], I16, kind="ExternalInput")
    wts_d = {nm: nc.dram_tensor(nm, [D, H], FP32, kind="ExternalInput")
             for nm in ("wat", "wbt", "w1t", "w2t", "w3t")}
    wlt_d = nc.dram_tensor("wlt", [H, 1], FP32, kind="ExternalInput")
    bias_d = {nm: nc.dram_tensor(nm, [2, 128, 1], FP32, kind="ExternalInput")
              for nm in ("ba", "bb", "b1", "b2", "b3")}
    bl_d = nc.dram_tensor("bl", [1, 1], FP32, kind="ExternalInput")
    beta_d = nc.dram_tensor("beta", [128, 1], FP32, kind="ExternalInput")
    ident_d = nc.dram_tensor("ident", [128, 128], FP32, kind="ExternalInput")
    out_d = nc.dram_tensor("out", [1, ec], FP32, kind="ExternalOutput")

    with tile.TileContext(nc) as tc:
        with (
            tc.tile_pool(name="const", bufs=1) as constp,
            tc.tile_pool(name="gath", bufs=4) as gathp,
            tc.tile_pool(name="work", bufs=2) as workp,
            tc.tile_pool(name="ghl", bufs=1) as ghlp,
            tc.tile_pool(name="acts", bufs=4) as actp,
            tc.tile_pool(name="px", bufs=2, space="PSUM") as pxp,
            tc.tile_pool(name="pt", bufs=2, space="PSUM") as ptp,
            tc.tile_pool(name="pm", bufs=2, space="PSUM") as pmp,
            tc.tile_pool(name="po", bufs=2, space="PSUM") as pop,
        ):
            # ---- persistent loads -------------------------------------
            # idx tiles FIRST: every gather waits on them, and HWDGE DMAs
            # execute in FIFO order per issuing engine
            idx0_sb = constp.tile([128, ec // 16], I16)
            nc.sync.dma_start(idx0_sb[:], idx0_d[:])
            idx1_sb = constp.tile([128, ec // 16], I16)
            nc.sync.dma_start(idx1_sb[:], idx1_d[:])
            idx0x_sb = constp.tile([128, ec // 16], I16)
            nc.sync.dma_start(idx0x_sb[:], idx0x_d[:])
            idx1x_sb = constp.tile([128, ec // 16], I16)
            nc.sync.dma_start(idx1x_sb[:], idx1x_d[:])

            # split the x table load so the first matmuls don't wait on
            # the whole 10 MB transfer
            n_xs = min(8, nch)
            cps = nch // n_xs              # chunks per x sub-tile
            xhl_t = []
            for i in range(n_xs):
                t = constp.tile([128, cps, 2 * D], BF16, tag=f"xhl{i}")
                nc.sync.dma_start(
                    t[:],
                    xhl_d[i * cps * 128:(i + 1) * cps * 128, :]
                    .rearrange("(c p) d -> p c d", p=128))
                xhl_t.append(t)

            def xhl_chunk(g):
                return xhl_t[g // cps][:, g % cps, :]

            w_sb = {}
            for nm, d in wts_d.items():
                t = constp.tile([128, 2, H], FP32, tag=f"w_{nm}")
                nc.sync.dma_start(t[:], d[:].rearrange("(k p) h -> p k h", p=128))
                w_sb[nm] = t
            wlt_sb = constp.tile([128, 2, 1], FP32)
            nc.sync.dma_start(wlt_sb[:], wlt_d[:].rearrange("(k p) o -> p k o", p=128))
            b_sb = {}
            for nm, d in bias_d.items():
                t = constp.tile([128, 2, 1], FP32, tag=f"b_{nm}")
                nc.sync.dma_start(t[:], d[:].rearrange("t p o -> p t o"))
                b_sb[nm] = t
            bl_sb = constp.tile([1, 1], FP32)
            nc.sync.dma_start(bl_sb[:], bl_d[:])
            beta_sb = constp.tile([128, 1], FP32)
            nc.sync.dma_start(beta_sb[:], beta_d[:])
            ident_sb = constp.tile([128, 128], FP32)
            nc.sync.dma_start(ident_sb[:], ident_d[:])

            xcnT = constp.tile([128, 2, ec], FP32)   # feature-major xcn
            out_sb = constp.tile([1, ec], FP32)

            # ---- MLP for one edge-half (ehw edges), feature-major fp32;
            # emitted mid-loop so its PE work overlaps gather-bound spans.
            def lin_h(src_ap, wname, bname, relu, dst):
                w, bias = w_sb[wname], b_sb[bname]
                for t in range(2):
                    pm = pmp.tile([128, ehw], FP32, tag="pm")
                    for k in range(2):
                        nc.tensor.matmul(
                            pm[:], w[:, k, t * 128:(t + 1) * 128],
                            src_ap[:, k, :], start=(k == 0), stop=(k == 1))
                    dsl = dst[:, t, :]
                    if t % 2 == 0:
                        nc.scalar.activation(
                            dsl, pm[:], AF.Relu if relu else AF.Identity,
                            bias=bias[:, t, :])
                    elif relu:
                        nc.vector.tensor_scalar(
                            dsl, pm[:], bias[:, t, :], 0.0,
                            mybir.AluOpType.add, mybir.AluOpType.max)
                    else:
                        nc.vector.tensor_scalar_add(dsl, pm[:], bias[:, t, :])
                return dst

            def mlp_half(hh):
                esl = slice(hh * ehw, (hh + 1) * ehw)
                xiT = actp.tile([128, 2, ehw], FP32, tag="act")
                xjT = actp.tile([128, 2, ehw], FP32, tag="act")
                for dst, isb in ((xiT, idx0x_sb), (xjT, idx1x_sb)):
                    ghl = ghlp.tile([128, 4, ehw], BF16, tag="ghl")
                    nc.gpsimd.dma_gather(
                        ghl[:], xhl_d[:],
                        isb[:, hh * (ehw // 16):(hh + 1) * (ehw // 16)],
                        ehw, ehw, elem_size=2 * D, transpose=True)
                    nc.vector.tensor_add(dst[:], ghl[:, 0:2, :], ghl[:, 2:4, :])
                pT = actp.tile([128, 2, ehw], FP32, tag="act")
                nc.vector.tensor_mul(pT[:], xiT[:], xjT[:])
                u = lin_h(pT[:], "wat", "ba", True,
                          actp.tile([128, 2, ehw], FP32, tag="act",
                                    name=f"u{hh}"))
                xijT = lin_h(u[:], "wbt", "bb", False,
                             actp.tile([128, 2, ehw], FP32, tag="act",
                                       name=f"xijT{hh}"))
                h = xcnT[:, :, esl]
                for li, (wn, bn, rl) in enumerate((
                        ("w1t", "b1", True), ("w2t", "b2", True),
                        ("w3t", "b3", False))):
                    h = lin_h(h, wn, bn, rl,
                              actp.tile([128, 2, ehw], FP32, tag="act",
                                        name=f"h{hh}_{li}"))[:]
                nc.vector.tensor_scalar_mul(h, h, beta_sb[:])
                nc.vector.tensor_add(h, h, xijT[:])
                po = pop.tile([1, ehw], FP32, tag="po")
                for k in range(2):
                    nc.tensor.matmul(po[:], wlt_sb[:, k, :], h[:, k, :],
                                     start=(k == 0), stop=(k == 1))
                nc.scalar.activation(out_sb[:, hh * ehw:(hh + 1) * ehw],
                                     po[:], AF.Identity, bias=bl_sb[:])

            # ---- main loop: cn^T and xcn ------------------------------
            for b in range(nb):
                px = pxp.tile([128, 2 * D], FP32, tag="px")
                for hf in range(n_halves):
                    a0 = gathp.tile([128, ch_h, EB], FP8, tag="a0")
                    a1 = gathp.tile([128, ch_h, EB], FP8, tag="a1")
                    col0 = hf * ch_h * 128
                    nc.gpsimd.dma_gather(
                        a0[:], adjb[:, col0:col0 + ch_h * 128],
                        idx0_sb[:, b * (EB // 16):(b + 1) * (EB // 16)],
                        EB, EB, elem_size=ch_h * 128, elem_step=npad,
                        transpose=True)
                    nc.gpsimd.dma_gather(
                        a1[:], adjb[:, col0:col0 + ch_h * 128],
                        idx1_sb[:, b * (EB // 16):(b + 1) * (EB // 16)],
                        EB, EB, elem_size=ch_h * 128, elem_step=npad,
                        transpose=True)
                    # cn = a0 AND a1 (binary fp8: bitwise AND == product),
                    # two fp8 lanes per u16 -> DVE 16-bit 2x mode.  Finer
                    # sub-ops so matmuls start before the whole half is done.
                    v0 = a0[:].rearrange("p a b -> p (a b)").bitcast(I16)
                    v1 = a1[:].rearrange("p a b -> p (a b)").bitcast(I16)
                    usz = ch_h * EB // 2
                    psz = usz // 4
                    for sp in range(0, usz, psz):
                        nc.vector.tensor_tensor(
                            v0[:, sp:sp + psz], v0[:, sp:sp + psz],
                            v1[:, sp:sp + psz], mybir.AluOpType.bitwise_and)
                    # fp8 transposed gathers interleave two n per partition:
                    # flat free index = c2*2*EB + 2*i + par holds node
                    # n = col0 + 2*(c2*128 + r) + par for edge i
                    a0v = a0[:].rearrange("p a b -> p (a b)").rearrange(
                        "p (c i two) -> p c two i", c=ch_h // 2, i=EB, two=2)
                    for c2 in range(ch_h // 2):
                        for par in range(2):
                            gp = hf * ch_h + 2 * c2 + par
                            nc.tensor.matmul(
                                px[:], a0v[:, c2, par, :], xhl_chunk(gp),
                                start=(gp == 0), stop=(gp == nch - 1))
                # TensorTensor may read at most one PSUM operand: stage the
                # [hi|lo] psum through SBUF, then add the halves on DVE.
                xcn_hl = workp.tile([128, 2 * D], FP32, tag="xcnhl")
                nc.scalar.activation(xcn_hl[:], px[:], AF.Copy)
                xcn = workp.tile([128, D], FP32, tag="xcn")
                nc.vector.tensor_add(xcn[:], xcn_hl[:, 0:D], xcn_hl[:, D:2 * D])
                pt = ptp.tile([128, 2, 128], FP32, tag="pt")
                for t in range(2):
                    nc.tensor.transpose(
                        pt[:, t, :], xcn[:, t * 128:(t + 1) * 128], ident_sb[:])
                    nc.scalar.activation(
                        xcnT[:, t, b * EB:(b + 1) * EB], pt[:, t, :], AF.Copy)
                if (b + 1) % (nb // n_eh) == 0:
                    mlp_half((b + 1) // (nb // n_eh) - 1)

            nc.sync.dma_start(out_d[:], out_sb[:])

    nc.compile()
    return nc


def _wrap_idx(e_slice, ec):
    """Pack indices for dma_gather: [128, ec//16] int16, idx i at
    [i % 16, i // 16], replicated over the 8 groups of 16 partitions."""
    a = np.asarray(e_slice).astype(np.int16)
    w = a.reshape(ec // 16, 16).T.copy()
    return np.ascontiguousarray(np.tile(w, (8, 1)))


def _interleave_perm(npad, n_halves=2):
    """Row permutation of the x table matching the fp8 gather interleave:
    table row gp*128 + r holds node n = hf*ch*128 + c2*256 + 2r + p, where
    gp = hf*ch + 2*c2 + p and ch = chunks per half."""
    nch = npad // 128
    ch = nch // n_halves
    gp = np.arange(nch)
    hf, rem = gp // ch, gp % ch
    base = hf * ch * 128 + (rem // 2) * 256 + rem % 2
    perm = (base[:, None] + 2 * np.arange(128)[None, :]).reshape(-1)
    inv = np.empty(npad, np.int64)
    inv[perm] = np.arange(npad)
    return perm, inv


def prepare_inputs(x, adj, edge, W1, b1, W2, b2, W3, b3, Wa, ba, Wb, bb,
                   Wl, bl, beta, n=N, npad=NPAD, ncores=N_CORES):
    x = np.asarray(x, np.float32)
    adj = np.asarray(adj, np.float32)
    edge = np.asarray(edge)
    ec = edge.shape[0] // ncores

    adjb = np.zeros((n, npad), FP8_NP)
    adjb[:, :n] = adj.astype(FP8_NP)

    x_hi = x.astype(BF16_NP)
    x_lo = (x - x_hi.astype(np.float32)).astype(BF16_NP)
    xhl = np.zeros((npad, 2 * D), BF16_NP)
    xhl[:n, :D] = x_hi
    xhl[:n, D:] = x_lo
    perm, inv = _interleave_perm(npad)
    xhl = np.ascontiguousarray(xhl[perm])

    common = dict(
        adjb=adjb, xhl=xhl,
        wat=np.ascontiguousarray(np.asarray(Wa, np.float32).T),
        wbt=np.ascontiguousarray(np.asarray(Wb, np.float32).T),
        w1t=np.ascontiguousarray(np.asarray(W1, np.float32).T),
        w2t=np.ascontiguousarray(np.asarray(W2, np.float32).T),
        w3t=np.ascontiguousarray(np.asarray(W3, np.float32).T),
        wlt=np.ascontiguousarray(np.asarray(Wl, np.float32).T),
        ba=np.asarray(ba, np.float32).reshape(2, 128, 1),
        bb=np.asarray(bb, np.float32).reshape(2, 128, 1),
        b1=np.asarray(b1, np.float32).reshape(2, 128, 1),
        b2=np.asarray(b2, np.float32).reshape(2, 128, 1),
        b3=np.asarray(b3, np.float32).reshape(2, 128, 1),
        bl=np.asarray(bl, np.float32).reshape(1, 1),
        beta=np.full((128, 1), np.asarray(beta, np.float32).reshape(-1)[0],
                     np.float32),
        ident=np.eye(128, dtype=np.float32),
    )
    in_maps = []
    for c in range(ncores):
        sl = slice(c * ec, (c + 1) * ec)
        m = dict(common)
        m["idx0"] = _wrap_idx(edge[sl, 0], ec)
        m["idx1"] = _wrap_idx(edge[sl, 1], ec)
        m["idx0x"] = _wrap_idx(inv[edge[sl, 0]], ec)
        m["idx1x"] = _wrap_idx(inv[edge[sl, 1]], ec)
        in_maps.append(m)
    return in_maps


_CACHE = {}


def _get_program():
    if "nc" not in _CACHE:
        _CACHE["nc"] = build_program()
    return _CACHE["nc"]


def run(in_maps, **kw):
    nc = _get_program()
    return run_bass_kernel_spmd(nc, in_maps, list(range(N_CORES)), **kw)


def kernel(**inputs):
    in_maps = prepare_inputs(**inputs)
    res = run(in_maps)
    out = np.concatenate([res.results[c]["out"][0] for c in range(N_CORES)])
    return out.reshape(E, 1).astype(np.float32)



# revision 10
# speedup vs baseline: 1.8623x; 1.8623x over previous
"""CNLP (common-neighbor link prediction) kernel for Trainium2, 8 NeuronCores.

Reference computation (per query edge e = (i, j)):
    cn  = adj[i] * adj[j]                      # common-neighbor indicator [N]
    xcn = cn @ x                               # sum of common-neighbor feats
    xij = relu(x[i]*x[j] @ Wa.T + ba) @ Wb.T + bb
    hcn = (relu->relu->lin) 3-layer MLP on xcn
    out = (hcn * beta + xij) @ Wl.T + bl       # [E, 1]

Sharding: edges (E=8192) split 8 x 1024 across cores; adj/x/weights replicated.

Device strategy per core (v2 — bitpacked adjacency + fp8 DoubleRow):
  - adj rows are BITPACKED on the host: [N, 1280] bytes (8 nodes/byte,
    little bit order).  gpsimd dma_gather(transpose=True) pulls the two
    packed rows per edge batch (128 edges) — 8x less HBM traffic than fp8.
  - DVE ANDs the packed rows (bitwise AND == product for binary rows),
    then extracts 8 bit-planes with one fused shift+mask tensor_scalar op
    per plane: each surviving bit sits at position 4 of its byte, i.e.
    fp8e4m3 value 2^-5; the matching x-table rows are pre-scaled by 2^5.
  - x is uploaded as a [10240, 512] fp8 table of [x_hi | x_lo] (hi/lo split
    of 32*x, so two e4m3 matmuls recover ~9-bit-mantissa accuracy), rows
    permuted to plane-major order so each (plane, byte-chunk) pair of
    128-node groups is contiguous.
  - TensorE runs fp8 DoubleRow matmuls (256-deep contraction per
    instruction, 0.5 PE cycles/row) with the x-table pairs as the
    STATIONARY operand (contiguous, satisfies the dual-fp8 Ldweights
    stride rules) and the cn planes as the moving operand: the output
    lands feature-major ([d, e] = xcn^T) so no PE transposes are needed;
    ACT+DVE fold the hi/lo halves while converting to bf16.
  - xi/xj are gathered from a separate natural-order bf16 [x_hi | x_lo]
    table (exact same path as v1, proven on HW).
  - MLPs run in bf16 (1 PE cycle/row), feature-major, biases+ReLU fused
    into ScalarE activations.  Output [1, 1024] fp32 per core.
"""

import numpy as np
import ml_dtypes

import concourse.bacc as bacc
import concourse.tile as tile
import concourse.mybir as mybir
from concourse.bass_utils import run_bass_kernel_spmd

BF16 = mybir.dt.bfloat16
FP32 = mybir.dt.float32
F32R = mybir.dt.float32r
FP8 = mybir.dt.float8e4
I16 = mybir.dt.int16
AF = mybir.ActivationFunctionType
ALU = mybir.AluOpType
DR = mybir.MatmulPerfMode.DoubleRow
FP8_NP = ml_dtypes.float8_e4m3

N_CORES = 8
N, E, D, H = 10000, 8192, 256, 256
NPAD = 10240                      # n padded to a multiple of 2048
PB = NPAD // 8                    # packed bytes per adjacency row (1280)
C2 = PB // 256                    # 256-byte chunks per row (5)
NPAIR = 8 * C2                    # DoubleRow matmuls per edge batch (40)
EC = E // N_CORES                 # 1024 edges per core
EB = 128                          # edges per batch (one gather tile)
XSCALE = 32.0                     # x table pre-scale (2^5, exact in fp8)


def build_program(npad=NPAD, ec=EC):
    nb = ec // EB                  # edge batches (8)
    ehw = min(512, ec)             # edge columns per MLP matmul
    n_eh = ec // ehw               # MLP halves (2)
    n_xs = 8                       # x table sub-tiles
    prs = NPAIR // n_xs            # pairs per x sub-tile (5)
    u16 = PB // 2                  # int16 lanes per packed row (640)

    nc = bacc.Bacc("TRN2", target_bir_lowering=False, debug=False,
                   enable_asserts=False, num_devices=N_CORES)

    adjp_d = nc.dram_tensor("adjp", [N, PB], FP8, kind="ExternalInput")
    xhl_d = nc.dram_tensor("xhl", [npad, 2 * D], FP8, kind="ExternalInput")
    xg_d = nc.dram_tensor("xg", [npad, 2 * D], BF16, kind="ExternalInput")
    # gather indices: raw node ids (adjacency rows + natural-order xg rows)
    idx0_d = nc.dram_tensor("idx0", [128, ec // 16], I16, kind="ExternalInput")
    idx1_d = nc.dram_tensor("idx1", [128, ec // 16], I16, kind="ExternalInput")
    wts_d = {nm: nc.dram_tensor(nm, [D, H], BF16, kind="ExternalInput")
             for nm in ("wat", "wbt", "w1t", "w2t", "w3t")}
    wlt_d = nc.dram_tensor("wlt", [H, 1], BF16, kind="ExternalInput")
    bias_d = {nm: nc.dram_tensor(nm, [2, 128, 1], FP32, kind="ExternalInput")
              for nm in ("ba", "bb", "b1", "b2", "b3")}
    bl_d = nc.dram_tensor("bl", [1, 1], FP32, kind="ExternalInput")
    beta_d = nc.dram_tensor("beta", [128, 1], FP32, kind="ExternalInput")
    out_d = nc.dram_tensor("out", [1, ec], FP32, kind="ExternalOutput")

    with tile.TileContext(nc) as tc:
        with (
            tc.tile_pool(name="const", bufs=1) as constp,
            tc.tile_pool(name="gath", bufs=2) as gathp,
            tc.tile_pool(name="plane", bufs=2) as planep,
            tc.tile_pool(name="work", bufs=2) as workp,
            tc.tile_pool(name="ghl", bufs=2) as ghlp,
            tc.tile_pool(name="acts", bufs=4) as actp,
            tc.tile_pool(name="px", bufs=1, space="PSUM") as pxp,
            tc.tile_pool(name="pm", bufs=2, space="PSUM") as pmp,
            tc.tile_pool(name="po", bufs=2, space="PSUM") as pop,
        ):
            # ---- persistent loads -------------------------------------
            # idx tiles FIRST: every gather waits on them, and HWDGE DMAs
            # execute in FIFO order per issuing engine
            idx0_sb = constp.tile([128, ec // 16], I16)
            nc.sync.dma_start(idx0_sb[:], idx0_d[:])
            idx1_sb = constp.tile([128, ec // 16], I16)
            nc.sync.dma_start(idx1_sb[:], idx1_d[:])

            # small weights next (first mlp_half lands ~1/2 through), then
            # the x table split in 8 so batch-0 matmuls stream behind it
            w_sb = {}
            for nm, d in wts_d.items():
                t = constp.tile([128, 2, H], BF16, tag=f"w_{nm}")
                nc.sync.dma_start(t[:], d[:].rearrange("(k p) h -> p k h", p=128))
                w_sb[nm] = t
            wlt_sb = constp.tile([128, 2, 1], BF16)
            nc.sync.dma_start(wlt_sb[:], wlt_d[:].rearrange("(k p) o -> p k o", p=128))
            b_sb = {}
            for nm, d in bias_d.items():
                t = constp.tile([128, 2, 1], FP32, tag=f"b_{nm}")
                nc.sync.dma_start(t[:], d[:].rearrange("t p o -> p t o"))
                b_sb[nm] = t
            bl_sb = constp.tile([1, 1], FP32)
            nc.sync.dma_start(bl_sb[:], bl_d[:])
            beta_sb = constp.tile([128, 1], FP32)
            nc.sync.dma_start(beta_sb[:], beta_d[:])

            # x table: [128, 2*prs, 512] per sub-tile; pair g rows live at
            # [g*256, (g+1)*256) = sub-tile g//prs, free rows 2*(g%prs)+par
            xhl_t = []
            for i in range(n_xs):
                t = constp.tile([128, 2 * prs, 2 * D], FP8, tag=f"xhl{i}")
                nc.sync.dma_start(
                    t[:],
                    xhl_d[i * prs * 256:(i + 1) * prs * 256, :]
                    .rearrange("(q p) d -> p q d", p=128))
                xhl_t.append(t)

            xcnT = constp.tile([128, 2, ec], BF16)   # feature-major xcn
            out_sb = constp.tile([1, ec], FP32)

            # ---- MLP for one edge-half (ehw edges), feature-major f32r;
            # emitted mid-loop so its PE work overlaps gather-bound spans.
            def lin_h(src_ap, wname, bname, relu, dst):
                w, bias = w_sb[wname], b_sb[bname]
                for t in range(2):
                    pm = pmp.tile([128, ehw], FP32, tag="pm")
                    for k in range(2):
                        nc.tensor.matmul(
                            pm[:], w[:, k, t * 128:(t + 1) * 128],
                            src_ap[:, k, :], start=(k == 0), stop=(k == 1))
                    dsl = dst[:, t, :]
                    if t % 2 == 0:
                        nc.scalar.activation(
                            dsl, pm[:], AF.Relu if relu else AF.Identity,
                            bias=bias[:, t, :])
                    elif relu:
                        nc.vector.tensor_scalar(
                            dsl, pm[:], bias[:, t, :], 0.0, ALU.add, ALU.max)
                    else:
                        nc.vector.tensor_scalar_add(dsl, pm[:], bias[:, t, :])
                return dst

            def mlp_half(hh):
                esl = slice(hh * ehw, (hh + 1) * ehw)
                xiT = actp.tile([128, 2, ehw], FP32, tag="act")
                xjT = actp.tile([128, 2, ehw], FP32, tag="act")
                for dst, isb in ((xiT, idx0_sb), (xjT, idx1_sb)):
                    ghl = ghlp.tile([128, 4, ehw], BF16, tag="ghl")
                    nc.gpsimd.dma_gather(
                        ghl[:], xg_d[:],
                        isb[:, hh * (ehw // 16):(hh + 1) * (ehw // 16)],
                        ehw, ehw, elem_size=2 * D, transpose=True)
                    nc.vector.tensor_add(dst[:], ghl[:, 0:2, :], ghl[:, 2:4, :])
                pT = actp.tile([128, 2, ehw], BF16, tag="actb")
                nc.vector.tensor_mul(pT[:], xiT[:], xjT[:])
                u = lin_h(pT[:], "wat", "ba", True,
                          actp.tile([128, 2, ehw], BF16, tag="actb",
                                    name=f"u{hh}"))
                xijT = lin_h(u[:], "wbt", "bb", False,
                             actp.tile([128, 2, ehw], BF16, tag="actb",
                                       name=f"xijT{hh}"))
                h = xcnT[:, :, esl]
                for li, (wn, bn, rl) in enumerate((
                        ("w1t", "b1", True), ("w2t", "b2", True),
                        ("w3t", "b3", False))):
                    h = lin_h(h, wn, bn, rl,
                              actp.tile([128, 2, ehw], BF16, tag="actb",
                                        name=f"h{hh}_{li}"))[:]
                nc.vector.tensor_scalar_mul(h, h, beta_sb[:])
                nc.vector.tensor_add(h, h, xijT[:])
                po = pop.tile([1, ehw], FP32, tag="po")
                for k in range(2):
                    nc.tensor.matmul(po[:], wlt_sb[:, k, :], h[:, k, :],
                                     start=(k == 0), stop=(k == 1))
                nc.scalar.activation(out_sb[:, hh * ehw:(hh + 1) * ehw],
                                     po[:], AF.Identity, bias=bl_sb[:])

            # ---- main loop: packed gather -> AND -> planes -> xcn^T ---
            for sb in range(n_eh):
                isl = slice(sb * (ehw // 16), (sb + 1) * (ehw // 16))
                g0 = gathp.tile([128, PB // 128, ehw], FP8, tag="g0")
                nc.gpsimd.dma_gather(g0[:], adjp_d[:], idx0_sb[:, isl],
                                     ehw, ehw, elem_size=PB, transpose=True)
                g1 = gathp.tile([128, PB // 128, ehw], FP8, tag="g1")
                nc.gpsimd.dma_gather(g1[:], adjp_d[:], idx1_sb[:, isl],
                                     ehw, ehw, elem_size=PB, transpose=True)
                # cn = g0 AND g1 on int16 lanes (two packed bytes per lane)
                v0 = g0[:].rearrange("p a b -> p (a b)").bitcast(I16)
                v1 = g1[:].rearrange("p a b -> p (a b)").bitcast(I16)
                usz = PB * ehw // 256
                nc.vector.tensor_tensor(v0, v0, v1, ALU.bitwise_and)
                # bit-plane extraction: plane k = (v >> (k-4)) & 0x1010,
                # one fused shift+mask op each; surviving bit = fp8 2^-5
                pl = planep.tile([128, 8, C2, ehw, 2], FP8, tag="pl")
                pli = pl[:].rearrange("p k c e t -> p (k c e t)").bitcast(I16)
                for k in range(8):
                    dstk = pli[:, k * usz:(k + 1) * usz]
                    if k < 4:
                        nc.vector.tensor_scalar(
                            dstk, v0, 4 - k, 0x1010,
                            ALU.logical_shift_left, ALU.bitwise_and)
                    elif k == 4:
                        nc.vector.tensor_scalar(
                            dstk, v0, 0x1010, 0, ALU.bitwise_and, ALU.bypass)
                    else:
                        nc.vector.tensor_scalar(
                            dstk, v0, k - 4, 0x1010,
                            ALU.logical_shift_right, ALU.bitwise_and)
                # 160 DoubleRow matmuls, x pairs stationary, planes moving:
                # px[c][d, e] += sum_par x[n, 128c+d] * cn[n, e]
                px = pxp.tile([128, 4, ehw], FP32, tag="px")
                for g in range(NPAIR):
                    k, c2 = g // C2, g % C2
                    rhs = pl[:, k, c2].rearrange("p e t -> p t e")
                    xp = xhl_t[g // prs]
                    lg = g % prs
                    for c in range(4):
                        nc.tensor.matmul(
                            px[:, c, :],
                            xp[:, 2 * lg:2 * lg + 2, 128 * c:128 * (c + 1)],
                            rhs, start=(g == 0), stop=(g == NPAIR - 1),
                            perf_mode=DR)
                # xcn^T = hi + lo halves (chunks c and c+2), bf16 out
                for t in range(2):
                    xcn_sb = workp.tile([128, ehw], FP32, tag="xcn")
                    nc.scalar.activation(xcn_sb[:], px[:, t, :], AF.Copy)
                    nc.vector.tensor_add(
                        xcnT[:, t, sb * ehw:(sb + 1) * ehw],
                        px[:, t + 2, :], xcn_sb[:])
                mlp_half(sb)

            nc.sync.dma_start(out_d[:], out_sb[:])

    nc.compile()
    return nc


def _wrap_idx(e_slice, ec):
    """Pack indices for dma_gather: [128, ec//16] int16, idx i at
    [i % 16, i // 16], replicated over the 8 groups of 16 partitions."""
    a = np.asarray(e_slice).astype(np.int16)
    w = a.reshape(ec // 16, 16).T.copy()
    return np.ascontiguousarray(np.tile(w, (8, 1)))


def _plane_row(npad=NPAD):
    """row_of_node[n]: x-table row for node n under the plane-major layout.
    Packed byte m = 256*c2 + 2*r + par of a gathered row lands at partition
    r, and bit k of that byte goes to plane k; the DoubleRow pair for
    (k, c2) contracts par=0,1, so node n = 8*m + k must live at table row
    ((k*C2 + c2)*2 + par)*128 + r."""
    n = np.arange(npad)
    k, m = n % 8, n // 8
    c2, rem = m // 256, m % 256
    r, par = rem // 2, rem % 2
    return ((k * C2 + c2) * 2 + par) * 128 + r


def prepare_inputs(x, adj, edge, W1, b1, W2, b2, W3, b3, Wa, ba, Wb, bb,
                   Wl, bl, beta, n=N, npad=NPAD, ncores=N_CORES):
    x = np.asarray(x, np.float32)
    adj = np.asarray(adj)
    edge = np.asarray(edge)
    ec = edge.shape[0] // ncores

    # bitpacked adjacency, little bit order: byte m bit k = adj[:, 8m+k]
    adjp = np.packbits(adj.astype(bool), axis=1, bitorder="little")
    if adjp.shape[1] < PB:
        adjp = np.pad(adjp, ((0, 0), (0, PB - adjp.shape[1])))
    adjp = np.ascontiguousarray(adjp).view(FP8_NP)

    # fp8 hi/lo split of 32*x, rows in plane-major order
    xs = XSCALE * x
    x_hi = xs.astype(FP8_NP)
    x_lo = (xs - x_hi.astype(np.float32)).astype(FP8_NP)
    xhl = np.zeros((npad, 2 * D), FP8_NP)
    row = _plane_row(npad)
    xhl[row[:n], :D] = x_hi
    xhl[row[:n], D:] = x_lo

    # bf16 hi/lo split of x, natural row order, for the xi/xj gathers
    xg_hi = x.astype(ml_dtypes.bfloat16)
    xg_lo = (x - xg_hi.astype(np.float32)).astype(ml_dtypes.bfloat16)
    xg = np.zeros((npad, 2 * D), ml_dtypes.bfloat16)
    xg[:n, :D] = xg_hi
    xg[:n, D:] = xg_lo

    common = dict(
        adjp=adjp, xhl=xhl, xg=xg,
        wat=np.ascontiguousarray(np.asarray(Wa, np.float32).T.astype(ml_dtypes.bfloat16)),
        wbt=np.ascontiguousarray(np.asarray(Wb, np.float32).T.astype(ml_dtypes.bfloat16)),
        w1t=np.ascontiguousarray(np.asarray(W1, np.float32).T.astype(ml_dtypes.bfloat16)),
        w2t=np.ascontiguousarray(np.asarray(W2, np.float32).T.astype(ml_dtypes.bfloat16)),
        w3t=np.ascontiguousarray(np.asarray(W3, np.float32).T.astype(ml_dtypes.bfloat16)),
        wlt=np.ascontiguousarray(np.asarray(Wl, np.float32).T.astype(ml_dtypes.bfloat16)),
        ba=np.asarray(ba, np.float32).reshape(2, 128, 1),
        bb=np.asarray(bb, np.float32).reshape(2, 128, 1),
        b1=np.asarray(b1, np.float32).reshape(2, 128, 1),
        b2=np.asarray(b2, np.float32).reshape(2, 128, 1),
        b3=np.asarray(b3, np.float32).reshape(2, 128, 1),
        bl=np.asarray(bl, np.float32).reshape(1, 1),
        beta=np.full((128, 1), np.asarray(beta, np.float32).reshape(-1)[0],
                     np.float32),
    )
    in_maps = []
    for c in range(ncores):
        sl = slice(c * ec, (c + 1) * ec)
        m = dict(common)
        m["idx0"] = _wrap_idx(edge[sl, 0], ec)
        m["idx1"] = _wrap_idx(edge[sl, 1], ec)
        in_maps.append(m)
    return in_maps


_CACHE = {}


def _get_program():
    if "nc" not in _CACHE:
        _CACHE["nc"] = build_program()
    return _CACHE["nc"]


def run(in_maps, **kw):
    nc = _get_program()
    return run_bass_kernel_spmd(nc, in_maps, list(range(N_CORES)), **kw)


def kernel(**inputs):
    in_maps = prepare_inputs(**inputs)
    res = run(in_maps)
    out = np.concatenate([res.results[c]["out"][0] for c in range(N_CORES)])
    return out.reshape(E, 1).astype(np.float32)


# revision 11
# speedup vs baseline: 2.1044x; 1.1300x over previous
"""CNLP (common-neighbor link prediction) kernel for Trainium2, 8 NeuronCores.

Reference computation (per query edge e = (i, j)):
    cn  = adj[i] * adj[j]                      # common-neighbor indicator [N]
    xcn = cn @ x                               # sum of common-neighbor feats
    xij = relu(x[i]*x[j] @ Wa.T + ba) @ Wb.T + bb
    hcn = (relu->relu->lin) 3-layer MLP on xcn
    out = (hcn * beta + xij) @ Wl.T + bl       # [E, 1]

Sharding: edges (E=8192) split 8 x 1024 across cores; adj/x/weights replicated.

Device strategy per core (v2 — bitpacked adjacency + fp8 DoubleRow):
  - adj rows are BITPACKED on the host: [N, 1280] bytes (8 nodes/byte,
    little bit order).  gpsimd dma_gather(transpose=True) pulls the two
    packed rows per edge batch (128 edges) — 8x less HBM traffic than fp8.
  - DVE ANDs the packed rows (bitwise AND == product for binary rows),
    then extracts 8 bit-planes with one fused shift+mask tensor_scalar op
    per plane: each surviving bit sits at position 4 of its byte, i.e.
    fp8e4m3 value 2^-5; the matching x-table rows are pre-scaled by 2^5.
  - x is uploaded as a [10240, 512] fp8 table of [x_hi | x_lo] (hi/lo split
    of 32*x, so two e4m3 matmuls recover ~9-bit-mantissa accuracy), rows
    permuted to plane-major order so each (plane, byte-chunk) pair of
    128-node groups is contiguous.
  - TensorE runs fp8 DoubleRow matmuls (256-deep contraction per
    instruction, 0.5 PE cycles/row) with the x-table pairs as the
    STATIONARY operand (contiguous, satisfies the dual-fp8 Ldweights
    stride rules) and the cn planes as the moving operand: the output
    lands feature-major ([d, e] = xcn^T) so no PE transposes are needed;
    ACT+DVE fold the hi/lo halves while converting to bf16.
  - xi/xj are gathered from a separate natural-order bf16 [x_hi | x_lo]
    table (exact same path as v1, proven on HW).
  - MLPs run in bf16 (1 PE cycle/row), feature-major, biases+ReLU fused
    into ScalarE activations.  Output [1, 1024] fp32 per core.
"""

import numpy as np
import ml_dtypes

import concourse.bacc as bacc
import concourse.tile as tile
import concourse.mybir as mybir
from concourse.bass_utils import run_bass_kernel_spmd

BF16 = mybir.dt.bfloat16
FP32 = mybir.dt.float32
F32R = mybir.dt.float32r
FP8 = mybir.dt.float8e4
I16 = mybir.dt.int16
AF = mybir.ActivationFunctionType
ALU = mybir.AluOpType
DR = mybir.MatmulPerfMode.DoubleRow
FP8_NP = ml_dtypes.float8_e4m3

N_CORES = 8
N, E, D, H = 10000, 8192, 256, 256
NPAD = 10240                      # n padded to a multiple of 2048
PB = NPAD // 8                    # packed bytes per adjacency row (1280)
C2 = PB // 256                    # 256-byte chunks per row (5)
NPAIR = 8 * C2                    # DoubleRow matmuls per edge batch (40)
EC = E // N_CORES                 # 1024 edges per core
EB = 128                          # edges per batch (one gather tile)
XSCALE = 32.0                     # x table pre-scale (2^5, exact in fp8)
NHALF = 1                         # 1 = single fp8 x table, 2 = [hi | lo]
XW = D * NHALF                    # fp8 x-table row width
NCH = XW // 128                   # output column chunks (2 or 4)


def build_program(npad=NPAD, ec=EC):
    nb = ec // EB                  # edge batches (8)
    ehw = min(512, ec)             # edge columns per MLP matmul
    n_eh = ec // ehw               # MLP halves (2)
    n_xs = 8                       # x table sub-tiles
    prs = NPAIR // n_xs            # pairs per x sub-tile (5)
    u16 = PB // 2                  # int16 lanes per packed row (640)

    nc = bacc.Bacc("TRN2", target_bir_lowering=False, debug=False,
                   enable_asserts=False, num_devices=N_CORES)

    adjp_d = nc.dram_tensor("adjp", [N, PB], FP8, kind="ExternalInput")
    xhl_d = nc.dram_tensor("xhl", [128, npad // 128, XW], FP8,
                           kind="ExternalInput")
    xg_d = nc.dram_tensor("xg", [npad, 2 * D], BF16, kind="ExternalInput")
    # gather indices: raw node ids (adjacency rows + natural-order xg rows)
    idx0_d = nc.dram_tensor("idx0", [128, ec // 16], I16, kind="ExternalInput")
    idx1_d = nc.dram_tensor("idx1", [128, ec // 16], I16, kind="ExternalInput")
    wts_d = {nm: nc.dram_tensor(nm, [D, H], BF16, kind="ExternalInput")
             for nm in ("wat", "wbt", "w1t", "w2t", "w3t")}
    wlt_d = nc.dram_tensor("wlt", [H, 1], BF16, kind="ExternalInput")
    bias_d = {nm: nc.dram_tensor(nm, [2, 128, 1], FP32, kind="ExternalInput")
              for nm in ("ba", "bb", "b1", "b2", "b3")}
    bl_d = nc.dram_tensor("bl", [1, 1], FP32, kind="ExternalInput")
    beta_d = nc.dram_tensor("beta", [128, 1], FP32, kind="ExternalInput")
    out_d = nc.dram_tensor("out", [1, ec], FP32, kind="ExternalOutput")

    with tile.TileContext(nc) as tc:
        with (
            tc.tile_pool(name="const", bufs=1) as constp,
            tc.tile_pool(name="gath", bufs=2) as gathp,
            tc.tile_pool(name="plane", bufs=2) as planep,
            tc.tile_pool(name="work", bufs=2) as workp,
            tc.tile_pool(name="ghl", bufs=2) as ghlp,
            tc.tile_pool(name="acts", bufs=4) as actp,
            tc.tile_pool(name="px", bufs=3 - NHALF, space="PSUM") as pxp,
            tc.tile_pool(name="pm", bufs=2, space="PSUM") as pmp,
            tc.tile_pool(name="po", bufs=2, space="PSUM") as pop,
        ):
            # ---- persistent loads -------------------------------------
            # idx tiles FIRST: every gather waits on them, and HWDGE DMAs
            # execute in FIFO order per issuing engine
            idx0_sb = constp.tile([128, ec // 16], I16)
            nc.sync.dma_start(idx0_sb[:], idx0_d[:])
            idx1_sb = constp.tile([128, ec // 16], I16)
            nc.sync.dma_start(idx1_sb[:], idx1_d[:])

            # x table next (the first super-batch's matmuls consume all
            # of it), pre-transposed on the host so each sub-tile load is
            # one contiguous descriptor per partition; weights later (the
            # first mlp_half lands ~1/3 through)
            xhl_t = []
            for i in range(n_xs):
                t = constp.tile([128, 2 * prs, XW], FP8, tag=f"xhl{i}")
                nc.sync.dma_start(t[:], xhl_d[:, i * 2 * prs:(i + 1) * 2 * prs, :])
                xhl_t.append(t)

            w_sb = {}
            for nm, d in wts_d.items():
                t = constp.tile([128, 2, H], BF16, tag=f"w_{nm}")
                nc.sync.dma_start(t[:], d[:].rearrange("(k p) h -> p k h", p=128))
                w_sb[nm] = t
            wlt_sb = constp.tile([128, 2, 1], BF16)
            nc.sync.dma_start(wlt_sb[:], wlt_d[:].rearrange("(k p) o -> p k o", p=128))
            b_sb = {}
            for nm, d in bias_d.items():
                t = constp.tile([128, 2, 1], FP32, tag=f"b_{nm}")
                nc.sync.dma_start(t[:], d[:].rearrange("t p o -> p t o"))
                b_sb[nm] = t
            bl_sb = constp.tile([1, 1], FP32)
            nc.sync.dma_start(bl_sb[:], bl_d[:])
            beta_sb = constp.tile([128, 1], FP32)
            nc.sync.dma_start(beta_sb[:], beta_d[:])

            xcnT = constp.tile([128, 2, ec], BF16)   # feature-major xcn
            out_sb = constp.tile([1, ec], FP32)

            # ---- MLP for one edge-half (ehw edges), feature-major f32r;
            # emitted mid-loop so its PE work overlaps gather-bound spans.
            def lin_h(src_ap, wname, bname, relu, dst):
                w, bias = w_sb[wname], b_sb[bname]
                for t in range(2):
                    pm = pmp.tile([128, ehw], FP32, tag="pm")
                    for k in range(2):
                        nc.tensor.matmul(
                            pm[:], w[:, k, t * 128:(t + 1) * 128],
                            src_ap[:, k, :], start=(k == 0), stop=(k == 1))
                    dsl = dst[:, t, :]
                    if t % 2 == 0:
                        nc.scalar.activation(
                            dsl, pm[:], AF.Relu if relu else AF.Identity,
                            bias=bias[:, t, :])
                    elif relu:
                        nc.vector.tensor_scalar(
                            dsl, pm[:], bias[:, t, :], 0.0, ALU.add, ALU.max)
                    else:
                        nc.vector.tensor_scalar_add(dsl, pm[:], bias[:, t, :])
                return dst

            def mlp_half(hh):
                esl = slice(hh * ehw, (hh + 1) * ehw)
                xiT = actp.tile([128, 2, ehw], FP32, tag="act")
                xjT = actp.tile([128, 2, ehw], FP32, tag="act")
                for dst, isb in ((xiT, idx0_sb), (xjT, idx1_sb)):
                    ghl = ghlp.tile([128, 4, ehw], BF16, tag="ghl")
                    nc.gpsimd.dma_gather(
                        ghl[:], xg_d[:],
                        isb[:, hh * (ehw // 16):(hh + 1) * (ehw // 16)],
                        ehw, ehw, elem_size=2 * D, transpose=True)
                    nc.vector.tensor_add(dst[:], ghl[:, 0:2, :], ghl[:, 2:4, :])
                pT = actp.tile([128, 2, ehw], BF16, tag="actb")
                nc.vector.tensor_mul(pT[:], xiT[:], xjT[:])
                u = lin_h(pT[:], "wat", "ba", True,
                          actp.tile([128, 2, ehw], BF16, tag="actb",
                                    name=f"u{hh}"))
                xijT = lin_h(u[:], "wbt", "bb", False,
                             actp.tile([128, 2, ehw], BF16, tag="actb",
                                       name=f"xijT{hh}"))
                h = xcnT[:, :, esl]
                for li, (wn, bn, rl) in enumerate((
                        ("w1t", "b1", True), ("w2t", "b2", True),
                        ("w3t", "b3", False))):
                    h = lin_h(h, wn, bn, rl,
                              actp.tile([128, 2, ehw], BF16, tag="actb",
                                        name=f"h{hh}_{li}"))[:]
                nc.vector.tensor_scalar_mul(h, h, beta_sb[:])
                nc.vector.tensor_add(h, h, xijT[:])
                po = pop.tile([1, ehw], FP32, tag="po")
                for k in range(2):
                    nc.tensor.matmul(po[:], wlt_sb[:, k, :], h[:, k, :],
                                     start=(k == 0), stop=(k == 1))
                nc.scalar.activation(out_sb[:, hh * ehw:(hh + 1) * ehw],
                                     po[:], AF.Identity, bias=bl_sb[:])

            # ---- main loop: packed gather -> AND -> planes -> xcn^T ---
            for sb in range(n_eh):
                isl = slice(sb * (ehw // 16), (sb + 1) * (ehw // 16))
                g0 = gathp.tile([128, PB // 128, ehw], FP8, tag="g0")
                nc.gpsimd.dma_gather(g0[:], adjp_d[:], idx0_sb[:, isl],
                                     ehw, ehw, elem_size=PB, transpose=True)
                g1 = gathp.tile([128, PB // 128, ehw], FP8, tag="g1")
                nc.gpsimd.dma_gather(g1[:], adjp_d[:], idx1_sb[:, isl],
                                     ehw, ehw, elem_size=PB, transpose=True)
                # cn = g0 AND g1 on int16 lanes (two packed bytes per lane)
                v0 = g0[:].rearrange("p a b -> p (a b)").bitcast(I16)
                v1 = g1[:].rearrange("p a b -> p (a b)").bitcast(I16)
                usz = PB * ehw // 256
                nc.vector.tensor_tensor(v0, v0, v1, ALU.bitwise_and)
                # bit-plane extraction: plane k = (v >> (k-4)) & 0x1010,
                # one fused shift+mask op each; surviving bit = fp8 2^-5
                pl = planep.tile([128, 8, C2, ehw, 2], FP8, tag="pl")
                pli = pl[:].rearrange("p k c e t -> p (k c e t)").bitcast(I16)
                for k in range(8):
                    dstk = pli[:, k * usz:(k + 1) * usz]
                    if k < 4:
                        nc.vector.tensor_scalar(
                            dstk, v0, 4 - k, 0x1010,
                            ALU.logical_shift_left, ALU.bitwise_and)
                    elif k == 4:
                        nc.vector.tensor_scalar(
                            dstk, v0, 0x1010, 0, ALU.bitwise_and, ALU.bypass)
                    else:
                        nc.vector.tensor_scalar(
                            dstk, v0, k - 4, 0x1010,
                            ALU.logical_shift_right, ALU.bitwise_and)
                # 160 DoubleRow matmuls, x pairs stationary, planes moving:
                # px[c][d, e] += sum_par x[n, 128c+d] * cn[n, e]
                px = pxp.tile([128, NCH, ehw], FP32, tag="px")
                for g in range(NPAIR):
                    k, c2 = g // C2, g % C2
                    rhs = pl[:, k, c2].rearrange("p e t -> p t e")
                    xp = xhl_t[g // prs]
                    lg = g % prs
                    for c in range(NCH):
                        nc.tensor.matmul(
                            px[:, c, :],
                            xp[:, 2 * lg:2 * lg + 2, 128 * c:128 * (c + 1)],
                            rhs, start=(g == 0), stop=(g == NPAIR - 1),
                            perf_mode=DR)
                # xcn^T to bf16 (adding the lo half first when NHALF == 2)
                for t in range(2):
                    dstT = xcnT[:, t, sb * ehw:(sb + 1) * ehw]
                    if NHALF == 2:
                        xcn_sb = workp.tile([128, ehw], FP32, tag="xcn")
                        nc.scalar.activation(xcn_sb[:], px[:, t, :], AF.Copy)
                        nc.vector.tensor_add(dstT, px[:, t + 2, :], xcn_sb[:])
                    else:
                        nc.scalar.activation(dstT, px[:, t, :], AF.Copy)
                mlp_half(sb)

            nc.sync.dma_start(out_d[:], out_sb[:])

    nc.compile()
    return nc


def _wrap_idx(e_slice, ec):
    """Pack indices for dma_gather: [128, ec//16] int16, idx i at
    [i % 16, i // 16], replicated over the 8 groups of 16 partitions."""
    a = np.asarray(e_slice).astype(np.int16)
    w = a.reshape(ec // 16, 16).T.copy()
    return np.ascontiguousarray(np.tile(w, (8, 1)))


def _plane_row(npad=NPAD):
    """row_of_node[n]: x-table row for node n under the plane-major layout.
    Packed byte m = 256*c2 + 2*r + par of a gathered row lands at partition
    r, and bit k of that byte goes to plane k; the DoubleRow pair for
    (k, c2) contracts par=0,1, so node n = 8*m + k must live at table row
    ((k*C2 + c2)*2 + par)*128 + r."""
    n = np.arange(npad)
    k, m = n % 8, n // 8
    c2, rem = m // 256, m % 256
    r, par = rem // 2, rem % 2
    return ((k * C2 + c2) * 2 + par) * 128 + r


def prepare_inputs(x, adj, edge, W1, b1, W2, b2, W3, b3, Wa, ba, Wb, bb,
                   Wl, bl, beta, n=N, npad=NPAD, ncores=N_CORES):
    x = np.asarray(x, np.float32)
    adj = np.asarray(adj)
    edge = np.asarray(edge)
    ec = edge.shape[0] // ncores

    # bitpacked adjacency, little bit order: byte m bit k = adj[:, 8m+k]
    adjp = np.packbits(adj.astype(bool), axis=1, bitorder="little")
    if adjp.shape[1] < PB:
        adjp = np.pad(adjp, ((0, 0), (0, PB - adjp.shape[1])))
    adjp = np.ascontiguousarray(adjp).view(FP8_NP)

    # fp8 split of 32*x ([hi | lo] when NHALF == 2), rows in plane-major
    # order, then pre-transposed to [partition, group, col] for the load
    xs = XSCALE * x
    x_hi = xs.astype(FP8_NP)
    xhl = np.zeros((npad, XW), FP8_NP)
    row = _plane_row(npad)
    xhl[row[:n], :D] = x_hi
    if NHALF == 2:
        xhl[row[:n], D:] = (xs - x_hi.astype(np.float32)).astype(FP8_NP)
    xhl = np.ascontiguousarray(
        xhl.reshape(npad // 128, 128, XW).transpose(1, 0, 2))

    # bf16 hi/lo split of x, natural row order, for the xi/xj gathers
    xg_hi = x.astype(ml_dtypes.bfloat16)
    xg_lo = (x - xg_hi.astype(np.float32)).astype(ml_dtypes.bfloat16)
    xg = np.zeros((npad, 2 * D), ml_dtypes.bfloat16)
    xg[:n, :D] = xg_hi
    xg[:n, D:] = xg_lo

    common = dict(
        adjp=adjp, xhl=xhl, xg=xg,
        wat=np.ascontiguousarray(np.asarray(Wa, np.float32).T.astype(ml_dtypes.bfloat16)),
        wbt=np.ascontiguousarray(np.asarray(Wb, np.float32).T.astype(ml_dtypes.bfloat16)),
        w1t=np.ascontiguousarray(np.asarray(W1, np.float32).T.astype(ml_dtypes.bfloat16)),
        w2t=np.ascontiguousarray(np.asarray(W2, np.float32).T.astype(ml_dtypes.bfloat16)),
        w3t=np.ascontiguousarray(np.asarray(W3, np.float32).T.astype(ml_dtypes.bfloat16)),
        wlt=np.ascontiguousarray(np.asarray(Wl, np.float32).T.astype(ml_dtypes.bfloat16)),
        ba=np.asarray(ba, np.float32).reshape(2, 128, 1),
        bb=np.asarray(bb, np.float32).reshape(2, 128, 1),
        b1=np.asarray(b1, np.float32).reshape(2, 128, 1),
        b2=np.asarray(b2, np.float32).reshape(2, 128, 1),
        b3=np.asarray(b3, np.float32).reshape(2, 128, 1),
        bl=np.asarray(bl, np.float32).reshape(1, 1),
        beta=np.full((128, 1), np.asarray(beta, np.float32).reshape(-1)[0],
                     np.float32),
    )
    in_maps = []
    for c in range(ncores):
        sl = slice(c * ec, (c + 1) * ec)
        m = dict(common)
        m["idx0"] = _wrap_idx(edge[sl, 0], ec)
        m["idx1"] = _wrap_idx(edge[sl, 1], ec)
        in_maps.append(m)
    return in_maps


_CACHE = {}


def _get_program():
    if "nc" not in _CACHE:
        _CACHE["nc"] = build_program()
    return _CACHE["nc"]


def run(in_maps, **kw):
    nc = _get_program()
    return run_bass_kernel_spmd(nc, in_maps, list(range(N_CORES)), **kw)


def kernel(**inputs):
    in_maps = prepare_inputs(**inputs)
    res = run(in_maps)
    out = np.concatenate([res.results[c]["out"][0] for c in range(N_CORES)])
    return out.reshape(E, 1).astype(np.float32)


# revision 17
# speedup vs baseline: 2.4789x; 1.1780x over previous
"""CNLP (common-neighbor link prediction) kernel for Trainium2, 8 NeuronCores.

Reference computation (per query edge e = (i, j)):
    cn  = adj[i] * adj[j]                      # common-neighbor indicator [N]
    xcn = cn @ x                               # sum of common-neighbor feats
    xij = relu(x[i]*x[j] @ Wa.T + ba) @ Wb.T + bb
    hcn = (relu->relu->lin) 3-layer MLP on xcn
    out = (hcn * beta + xij) @ Wl.T + bl       # [E, 1]

Sharding: edges (E=8192) split 8 x 1024 across cores; adj/x/weights replicated.

Device strategy per core (v2 — bitpacked adjacency + fp8 DoubleRow):
  - adj rows are BITPACKED on the host: [N, 1280] bytes (8 nodes/byte,
    little bit order).  gpsimd dma_gather(transpose=True) pulls the two
    packed rows per edge batch (128 edges) — 8x less HBM traffic than fp8.
  - DVE ANDs the packed rows (bitwise AND == product for binary rows),
    then extracts 8 bit-planes with one fused shift+mask tensor_scalar op
    per plane: each surviving bit sits at position 4 of its byte, i.e.
    fp8e4m3 value 2^-5; the matching x-table rows are pre-scaled by 2^5.
  - x is uploaded as a [10240, 512] fp8 table of [x_hi | x_lo] (hi/lo split
    of 32*x, so two e4m3 matmuls recover ~9-bit-mantissa accuracy), rows
    permuted to plane-major order so each (plane, byte-chunk) pair of
    128-node groups is contiguous.
  - TensorE runs fp8 DoubleRow matmuls (256-deep contraction per
    instruction, 0.5 PE cycles/row) with the x-table pairs as the
    STATIONARY operand (contiguous, satisfies the dual-fp8 Ldweights
    stride rules) and the cn planes as the moving operand: the output
    lands feature-major ([d, e] = xcn^T) so no PE transposes are needed;
    ACT+DVE fold the hi/lo halves while converting to bf16.
  - xi/xj are gathered from a separate natural-order bf16 [x_hi | x_lo]
    table (exact same path as v1, proven on HW).
  - MLPs run in bf16 (1 PE cycle/row), feature-major, biases+ReLU fused
    into ScalarE activations.  Output [1, 1024] fp32 per core.
"""

import numpy as np
import ml_dtypes

import concourse.bacc as bacc
import concourse.tile as tile
import concourse.mybir as mybir
from concourse.bass_utils import run_bass_kernel_spmd

BF16 = mybir.dt.bfloat16
FP32 = mybir.dt.float32
F32R = mybir.dt.float32r
FP8 = mybir.dt.float8e4
I16 = mybir.dt.int16
AF = mybir.ActivationFunctionType
ALU = mybir.AluOpType
DR = mybir.MatmulPerfMode.DoubleRow
FP8_NP = ml_dtypes.float8_e4m3

N_CORES = 8
N, E, D, H = 10000, 8192, 256, 256
NPAD = 10240                      # n padded to a multiple of 2048
PB = NPAD // 8                    # packed bytes per adjacency row (1280)
C2 = PB // 256                    # 256-byte chunks per row (5)
NPAIR = 8 * C2                    # DoubleRow matmuls per edge batch (40)
EC = E // N_CORES                 # 1024 edges per core
EB = 128                          # edges per batch (one gather tile)
XSCALE = 32.0                     # x table pre-scale (2^5, exact in fp8)
NHALF = 1                         # 1 = single fp8 x table, 2 = [hi | lo]
XW = D * NHALF                    # fp8 x-table row width
NCH = XW // 128                   # output column chunks (2 or 4)


def build_program(npad=NPAD, ec=EC):
    nb = ec // EB                  # edge batches (8)
    ehw = min(512, ec)             # edge columns per MLP matmul
    n_eh = ec // ehw               # MLP halves (2)
    n_xs = 8                       # x table sub-tiles
    prs = NPAIR // n_xs            # pairs per x sub-tile (5)
    u16 = PB // 2                  # int16 lanes per packed row (640)

    nc = bacc.Bacc("TRN2", target_bir_lowering=False, debug=False,
                   enable_asserts=False, num_devices=N_CORES)

    adjp_d = nc.dram_tensor("adjp", [N, PB], FP8, kind="ExternalInput")
    xhl_d = nc.dram_tensor("xhl", [128, npad // 128, XW], FP8,
                           kind="ExternalInput")
    xg_d = nc.dram_tensor("xg", [npad, 2 * D], BF16, kind="ExternalInput")
    # gather indices: raw node ids (adjacency rows + natural-order xg rows)
    idx0_d = nc.dram_tensor("idx0", [128, ec // 16], I16, kind="ExternalInput")
    idx1_d = nc.dram_tensor("idx1", [128, ec // 16], I16, kind="ExternalInput")
    idxs_d = nc.dram_tensor("idxs", [128, 8], I16, kind="ExternalInput")
    # all MLP weights pre-transposed to [partition, k-chunk, h] and packed
    # into one tensor (one contiguous descriptor per partition); biases+beta
    # likewise packed as fp32
    wpack_d = nc.dram_tensor("wpack", [128, 2688], BF16,
                             kind="ExternalInput")
    bpack_d = nc.dram_tensor("bpack", [128, 11], FP32, kind="ExternalInput")
    bl_d = nc.dram_tensor("bl", [1, 1], FP32, kind="ExternalInput")
    out_d = nc.dram_tensor("out", [1, ec], FP32, kind="ExternalOutput")

    with tile.TileContext(nc) as tc:
        with (
            tc.tile_pool(name="const", bufs=1) as constp,
            tc.tile_pool(name="gath", bufs=1) as gathp,
            tc.tile_pool(name="plane", bufs=2) as planep,
            tc.tile_pool(name="work", bufs=2) as workp,
            tc.tile_pool(name="ghl", bufs=1) as ghlp,
            tc.tile_pool(name="acts", bufs=4) as actp,
            tc.tile_pool(name="px", bufs=3 - NHALF, space="PSUM") as pxp,
            tc.tile_pool(name="pm", bufs=2, space="PSUM") as pmp,
            tc.tile_pool(name="po", bufs=2, space="PSUM") as pop,
        ):
            # ---- persistent loads -------------------------------------
            # idx tiles FIRST: every gather waits on them, and HWDGE DMAs
            # execute in FIFO order per issuing engine
            idx0_sb = constp.tile([128, ec // 16], I16)
            nc.sync.dma_start(idx0_sb[:], idx0_d[:])
            idx1_sb = constp.tile([128, ec // 16], I16)
            nc.sync.dma_start(idx1_sb[:], idx1_d[:])

            idxs_sb = constp.tile([128, 8], I16)
            nc.sync.dma_start(idxs_sb[:], idxs_d[:])
            bpack = constp.tile([128, 11], FP32)
            nc.sync.dma_start(bpack[:], bpack_d[:])
            b_sb = {nm: bpack[:, i * 2:(i + 1) * 2]
                    .rearrange("p (t o) -> p t o", t=2)
                    for i, nm in enumerate(("ba", "bb", "b1", "b2", "b3"))}
            beta_sb = bpack[:, 10:11]
            bl_sb = constp.tile([1, 1], FP32)
            nc.sync.dma_start(bl_sb[:], bl_d[:])

            # The first dma_gather only starts once every pending HWDGE
            # descriptor has drained, so the 2.6MB x table and the weights
            # are loaded THROUGH the SWDGE gather path instead (sequential
            # indices, one whole partition-row per index) — the HWDGE rings
            # then only ever hold the tiny idx/bias loads and the gather
            # pipeline starts ~10us earlier.
            xall = constp.tile([128, NPAIR * 2, XW], FP8)
            nc.gpsimd.dma_gather(
                xall[:].rearrange("p q d -> p (q d)").rearrange(
                    "p (o f) -> p o f", o=1),
                xhl_d[:].rearrange("p q d -> p (q d)"), idxs_sb[:],
                128, 128, elem_size=NPAIR * 2 * XW)
            xhl_t = [xall[:, i * 2 * prs:(i + 1) * 2 * prs, :]
                     for i in range(n_xs)]
            wpack = constp.tile([128, 2688], BF16)
            nc.gpsimd.dma_gather(
                wpack[:].rearrange("p (o f) -> p o f", o=1),
                wpack_d[:], idxs_sb[:], 128, 128, elem_size=2688)
            w_sb = {nm: wpack[:, i * 2 * H:(i + 1) * 2 * H]
                    .rearrange("p (k h) -> p k h", k=2)
                    for i, nm in enumerate(("wat", "wbt", "w1t", "w2t", "w3t"))}
            wlt_sb = wpack[:, 10 * H:10 * H + 2].rearrange(
                "p (k o) -> p k o", k=2)

            # adjacency + xi/xj gathers, all hoisted (gen serializes on
            # the gpsimd engine)
            gt = {}
            for s in range(n_eh):
                isl = slice(s * (ehw // 16), (s + 1) * (ehw // 16))
                for nm, isb in (("g0", idx0_sb), ("g1", idx1_sb)):
                    g = gathp.tile([128, PB // 128, ehw], FP8,
                                   tag=f"{nm}_{s}")
                    nc.gpsimd.dma_gather(g[:], adjp_d[:], isb[:, isl],
                                         ehw, ehw, elem_size=PB,
                                         transpose=True)
                    gt[nm, s] = g
            ghl_t = {}
            for s in range(n_eh):
                isl = slice(s * (ehw // 16), (s + 1) * (ehw // 16))
                for nm, isb in (("xi", idx0_sb), ("xj", idx1_sb)):
                    ghl = ghlp.tile([128, 4, ehw], BF16, tag=f"ghl{nm}{s}")
                    nc.gpsimd.dma_gather(ghl[:], xg_d[:], isb[:, isl],
                                         ehw, ehw, elem_size=2 * D,
                                         transpose=True)
                    ghl_t[nm, s] = ghl

            xcnT = constp.tile([128, 2, ec], BF16)   # feature-major xcn
            out_sb = constp.tile([1, ec], FP32)

            # ---- MLP for one edge-half (ehw edges), feature-major f32r;
            # emitted mid-loop so its PE work overlaps gather-bound spans.
            def lin_h(src_ap, wname, bname, relu, dst):
                w, bias = w_sb[wname], b_sb[bname]
                for t in range(2):
                    pm = pmp.tile([128, ehw], FP32, tag="pm")
                    for k in range(2):
                        nc.tensor.matmul(
                            pm[:], w[:, k, t * 128:(t + 1) * 128],
                            src_ap[:, k, :], start=(k == 0), stop=(k == 1))
                    dsl = dst[:, t, :]
                    if t % 2 == 0:
                        nc.scalar.activation(
                            dsl, pm[:], AF.Relu if relu else AF.Identity,
                            bias=bias[:, t, :])
                    elif relu:
                        nc.vector.tensor_scalar(
                            dsl, pm[:], bias[:, t, :], 0.0, ALU.add, ALU.max)
                    else:
                        nc.vector.tensor_scalar_add(dsl, pm[:], bias[:, t, :])
                return dst

            def mlp_half(hh):
                esl = slice(hh * ehw, (hh + 1) * ehw)
                xiT = actp.tile([128, 2, ehw], FP32, tag="act")
                xjT = actp.tile([128, 2, ehw], FP32, tag="act")
                for dst, nm in ((xiT, "xi"), (xjT, "xj")):
                    ghl = ghl_t[nm, hh]
                    nc.vector.tensor_add(dst[:], ghl[:, 0:2, :], ghl[:, 2:4, :])
                pT = actp.tile([128, 2, ehw], BF16, tag="actb")
                nc.vector.tensor_mul(pT[:], xiT[:], xjT[:])
                u = lin_h(pT[:], "wat", "ba", True,
                          actp.tile([128, 2, ehw], BF16, tag="actb",
                                    name=f"u{hh}"))
                xijT = lin_h(u[:], "wbt", "bb", False,
                             actp.tile([128, 2, ehw], BF16, tag="actb",
                                       name=f"xijT{hh}"))
                h = xcnT[:, :, esl]
                for li, (wn, bn, rl) in enumerate((
                        ("w1t", "b1", True), ("w2t", "b2", True),
                        ("w3t", "b3", False))):
                    h = lin_h(h, wn, bn, rl,
                              actp.tile([128, 2, ehw], BF16, tag="actb",
                                        name=f"h{hh}_{li}"))[:]
                nc.vector.tensor_scalar_mul(h, h, beta_sb[:])
                nc.vector.tensor_add(h, h, xijT[:])
                po = pop.tile([1, ehw], FP32, tag="po")
                for k in range(2):
                    nc.tensor.matmul(po[:], wlt_sb[:, k, :], h[:, k, :],
                                     start=(k == 0), stop=(k == 1))
                nc.scalar.activation(out_sb[:, hh * ehw:(hh + 1) * ehw],
                                     po[:], AF.Identity, bias=bl_sb[:])

            # ---- main loop: packed gather -> AND -> planes -> xcn^T ---
            for sb in range(n_eh):
                g0, g1 = gt["g0", sb], gt["g1", sb]
                # cn = g0 AND g1 on int16 lanes (two packed bytes per lane)
                v0 = g0[:].rearrange("p a b -> p (a b)").bitcast(I16)
                v1 = g1[:].rearrange("p a b -> p (a b)").bitcast(I16)
                usz = PB * ehw // 256
                nc.vector.tensor_tensor(v0, v0, v1, ALU.bitwise_and)
                # bit-plane extraction: plane k = (v >> (k-4)) & 0x1010,
                # one fused shift+mask op each; surviving bit = fp8 2^-5
                pl = planep.tile([128, 8, C2, ehw, 2], FP8, tag="pl")
                pli = pl[:].rearrange("p k c e t -> p (k c e t)").bitcast(I16)
                for k in range(8):
                    dstk = pli[:, k * usz:(k + 1) * usz]
                    if k < 4:
                        nc.vector.tensor_scalar(
                            dstk, v0, 4 - k, 0x1010,
                            ALU.logical_shift_left, ALU.bitwise_and)
                    elif k == 4:
                        nc.vector.tensor_scalar(
                            dstk, v0, 0x1010, 0, ALU.bitwise_and, ALU.bypass)
                    else:
                        nc.vector.tensor_scalar(
                            dstk, v0, k - 4, 0x1010,
                            ALU.logical_shift_right, ALU.bitwise_and)
                # 160 DoubleRow matmuls, x pairs stationary, planes moving:
                # px[c][d, e] += sum_par x[n, 128c+d] * cn[n, e]
                px = pxp.tile([128, NCH, ehw], FP32, tag="px")
                for g in range(NPAIR):
                    k, c2 = g // C2, g % C2
                    rhs = pl[:, k, c2].rearrange("p e t -> p t e")
                    xp = xhl_t[g // prs]
                    lg = g % prs
                    for c in range(NCH):
                        nc.tensor.matmul(
                            px[:, c, :],
                            xp[:, 2 * lg:2 * lg + 2, 128 * c:128 * (c + 1)],
                            rhs, start=(g == 0), stop=(g == NPAIR - 1),
                            perf_mode=DR)
                # xcn^T to bf16 (adding the lo half first when NHALF == 2)
                for t in range(2):
                    dstT = xcnT[:, t, sb * ehw:(sb + 1) * ehw]
                    if NHALF == 2:
                        xcn_sb = workp.tile([128, ehw], FP32, tag="xcn")
                        nc.scalar.activation(xcn_sb[:], px[:, t, :], AF.Copy)
                        nc.vector.tensor_add(dstT, px[:, t + 2, :], xcn_sb[:])
                    else:
                        nc.scalar.activation(dstT, px[:, t, :], AF.Copy)
                mlp_half(sb)

            nc.sync.dma_start(out_d[:], out_sb[:])

    nc.compile()
    return nc


def _wrap_idx(e_slice, ec):
    """Pack indices for dma_gather: [128, ec//16] int16, idx i at
    [i % 16, i // 16], replicated over the 8 groups of 16 partitions."""
    a = np.asarray(e_slice).astype(np.int16)
    w = a.reshape(ec // 16, 16).T.copy()
    return np.ascontiguousarray(np.tile(w, (8, 1)))


def _plane_row(npad=NPAD):
    """row_of_node[n]: x-table row for node n under the plane-major layout.
    Packed byte m = 256*c2 + 2*r + par of a gathered row lands at partition
    r, and bit k of that byte goes to plane k; the DoubleRow pair for
    (k, c2) contracts par=0,1, so node n = 8*m + k must live at table row
    ((k*C2 + c2)*2 + par)*128 + r."""
    n = np.arange(npad)
    k, m = n % 8, n // 8
    c2, rem = m // 256, m % 256
    r, par = rem // 2, rem % 2
    return ((k * C2 + c2) * 2 + par) * 128 + r


def prepare_inputs(x, adj, edge, W1, b1, W2, b2, W3, b3, Wa, ba, Wb, bb,
                   Wl, bl, beta, n=N, npad=NPAD, ncores=N_CORES):
    x = np.asarray(x, np.float32)
    adj = np.asarray(adj)
    edge = np.asarray(edge)
    ec = edge.shape[0] // ncores

    # bitpacked adjacency, little bit order: byte m bit k = adj[:, 8m+k]
    adjp = np.packbits(adj.astype(bool), axis=1, bitorder="little")
    if adjp.shape[1] < PB:
        adjp = np.pad(adjp, ((0, 0), (0, PB - adjp.shape[1])))
    adjp = np.ascontiguousarray(adjp).view(FP8_NP)

    # fp8 split of 32*x ([hi | lo] when NHALF == 2), rows in plane-major
    # order, then pre-transposed to [partition, group, col] for the load
    xs = XSCALE * x
    x_hi = xs.astype(FP8_NP)
    xhl = np.zeros((npad, XW), FP8_NP)
    row = _plane_row(npad)
    xhl[row[:n], :D] = x_hi
    if NHALF == 2:
        xhl[row[:n], D:] = (xs - x_hi.astype(np.float32)).astype(FP8_NP)
    xhl = np.ascontiguousarray(
        xhl.reshape(npad // 128, 128, XW).transpose(1, 0, 2))

    # bf16 hi/lo split of x, natural row order, for the xi/xj gathers
    xg_hi = x.astype(ml_dtypes.bfloat16)
    xg_lo = (x - xg_hi.astype(np.float32)).astype(ml_dtypes.bfloat16)
    xg = np.zeros((npad, 2 * D), ml_dtypes.bfloat16)
    xg[:n, :D] = xg_hi
    xg[:n, D:] = xg_lo

    # wpack[p, i*512:(i+1)*512] = W_i.T[(k p)] -> [p, (k h)]; +wlt at the end
    BF = ml_dtypes.bfloat16
    wpack = np.zeros((128, 2688), BF)
    for i, W in enumerate((Wa, Wb, W1, W2, W3)):
        wt = np.asarray(W, np.float32).T.astype(BF)      # [D, H], f = 128k + p
        wpack[:, i * 2 * H:(i + 1) * 2 * H] = \
            wt.reshape(2, 128, H).transpose(1, 0, 2).reshape(128, 2 * H)
    wlt = np.asarray(Wl, np.float32).T.astype(BF)        # [H, 1]
    wpack[:, 10 * H:10 * H + 2] = wlt.reshape(2, 128).T
    bpack = np.zeros((128, 11), np.float32)
    for i, b in enumerate((ba, bb, b1, b2, b3)):
        bpack[:, i * 2:(i + 1) * 2] = np.asarray(b, np.float32).reshape(2, 128).T
    bpack[:, 10] = np.asarray(beta, np.float32).reshape(-1)[0]

    common = dict(
        adjp=adjp, xhl=xhl, xg=xg, wpack=wpack, bpack=bpack,
        bl=np.asarray(bl, np.float32).reshape(1, 1),
        idxs=_wrap_idx(np.arange(128), 128),
    )
    in_maps = []
    for c in range(ncores):
        sl = slice(c * ec, (c + 1) * ec)
        m = dict(common)
        m["idx0"] = _wrap_idx(edge[sl, 0], ec)
        m["idx1"] = _wrap_idx(edge[sl, 1], ec)
        in_maps.append(m)
    return in_maps


_CACHE = {}


def _get_program():
    if "nc" not in _CACHE:
        _CACHE["nc"] = build_program()
    return _CACHE["nc"]


def run(in_maps, **kw):
    nc = _get_program()
    return run_bass_kernel_spmd(nc, in_maps, list(range(N_CORES)), **kw)


def kernel(**inputs):
    in_maps = prepare_inputs(**inputs)
    res = run(in_maps)
    out = np.concatenate([res.results[c]["out"][0] for c in range(N_CORES)])
    return out.reshape(E, 1).astype(np.float32)


# revision 18
# speedup vs baseline: 2.5584x; 1.0320x over previous
"""CNLP (common-neighbor link prediction) kernel for Trainium2, 8 NeuronCores.

Reference computation (per query edge e = (i, j)):
    cn  = adj[i] * adj[j]                      # common-neighbor indicator [N]
    xcn = cn @ x                               # sum of common-neighbor feats
    xij = relu(x[i]*x[j] @ Wa.T + ba) @ Wb.T + bb
    hcn = (relu->relu->lin) 3-layer MLP on xcn
    out = (hcn * beta + xij) @ Wl.T + bl       # [E, 1]

Sharding: edges (E=8192) split 8 x 1024 across cores; adj/x/weights replicated.

Device strategy per core (v2 — bitpacked adjacency + fp8 DoubleRow):
  - adj rows are BITPACKED on the host: [N, 1280] bytes (8 nodes/byte,
    little bit order).  gpsimd dma_gather(transpose=True) pulls the two
    packed rows per edge batch (128 edges) — 8x less HBM traffic than fp8.
  - DVE ANDs the packed rows (bitwise AND == product for binary rows),
    then extracts 8 bit-planes with one fused shift+mask tensor_scalar op
    per plane: each surviving bit sits at position 4 of its byte, i.e.
    fp8e4m3 value 2^-5; the matching x-table rows are pre-scaled by 2^5.
  - x is uploaded as a [10240, 512] fp8 table of [x_hi | x_lo] (hi/lo split
    of 32*x, so two e4m3 matmuls recover ~9-bit-mantissa accuracy), rows
    permuted to plane-major order so each (plane, byte-chunk) pair of
    128-node groups is contiguous.
  - TensorE runs fp8 DoubleRow matmuls (256-deep contraction per
    instruction, 0.5 PE cycles/row) with the x-table pairs as the
    STATIONARY operand (contiguous, satisfies the dual-fp8 Ldweights
    stride rules) and the cn planes as the moving operand: the output
    lands feature-major ([d, e] = xcn^T) so no PE transposes are needed;
    ACT+DVE fold the hi/lo halves while converting to bf16.
  - xi/xj are gathered from a separate natural-order bf16 [x_hi | x_lo]
    table (exact same path as v1, proven on HW).
  - MLPs run in bf16 (1 PE cycle/row), feature-major, biases+ReLU fused
    into ScalarE activations.  Output [1, 1024] fp32 per core.
"""

import numpy as np
import ml_dtypes

import concourse.bacc as bacc
import concourse.tile as tile
import concourse.mybir as mybir
from concourse.bass_utils import run_bass_kernel_spmd

BF16 = mybir.dt.bfloat16
FP32 = mybir.dt.float32
F32R = mybir.dt.float32r
FP8 = mybir.dt.float8e4
I16 = mybir.dt.int16
AF = mybir.ActivationFunctionType
ALU = mybir.AluOpType
DR = mybir.MatmulPerfMode.DoubleRow
FP8_NP = ml_dtypes.float8_e4m3

N_CORES = 8
N, E, D, H = 10000, 8192, 256, 256
NPAD = 10240                      # n padded to a multiple of 2048
PB = NPAD // 8                    # packed bytes per adjacency row (1280)
C2 = PB // 256                    # 256-byte chunks per row (5)
NPAIR = 8 * C2                    # DoubleRow matmuls per edge batch (40)
EC = E // N_CORES                 # 1024 edges per core
EB = 128                          # edges per batch (one gather tile)
XSCALE = 32.0                     # x table pre-scale (2^5, exact in fp8)
NHALF = 1                         # 1 = single fp8 x table, 2 = [hi | lo]
XW = D * NHALF                    # fp8 x-table row width
NCH = XW // 128                   # output column chunks (2 or 4)


def build_program(npad=NPAD, ec=EC):
    nb = ec // EB                  # edge batches (8)
    ehw = min(512, ec)             # edge columns per MLP matmul
    n_eh = ec // ehw               # MLP halves (2)
    n_xs = 8                       # x table sub-tiles
    prs = NPAIR // n_xs            # pairs per x sub-tile (5)
    u16 = PB // 2                  # int16 lanes per packed row (640)

    nc = bacc.Bacc("TRN2", target_bir_lowering=False, debug=False,
                   enable_asserts=False, num_devices=N_CORES)

    adjp_d = nc.dram_tensor("adjp", [N, PB], FP8, kind="ExternalInput")
    xhl_d = nc.dram_tensor("xhl", [128, npad // 128, XW], FP8,
                           kind="ExternalInput")
    xg_d = nc.dram_tensor("xg", [npad, 2 * D], BF16, kind="ExternalInput")
    # gather indices: raw node ids (adjacency rows + natural-order xg rows)
    idx0_d = nc.dram_tensor("idx0", [128, ec // 16], I16, kind="ExternalInput")
    idx1_d = nc.dram_tensor("idx1", [128, ec // 16], I16, kind="ExternalInput")
    # all MLP weights pre-transposed to [partition, k-chunk, h] and packed
    # into one tensor (one contiguous descriptor per partition); biases+beta
    # likewise packed as fp32
    wpack_d = nc.dram_tensor("wpack", [128, 5 * 2 * H + 2], BF16,
                             kind="ExternalInput")
    bpack_d = nc.dram_tensor("bpack", [128, 11], FP32, kind="ExternalInput")
    bl_d = nc.dram_tensor("bl", [1, 1], FP32, kind="ExternalInput")
    out_d = nc.dram_tensor("out", [1, ec], FP32, kind="ExternalOutput")

    with tile.TileContext(nc) as tc:
        with (
            tc.tile_pool(name="const", bufs=1) as constp,
            tc.tile_pool(name="gath", bufs=1) as gathp,
            tc.tile_pool(name="plane", bufs=2) as planep,
            tc.tile_pool(name="work", bufs=2) as workp,
            tc.tile_pool(name="ghl", bufs=1) as ghlp,
            tc.tile_pool(name="acts", bufs=4) as actp,
            tc.tile_pool(name="px", bufs=3 - NHALF, space="PSUM") as pxp,
            tc.tile_pool(name="pm", bufs=2, space="PSUM") as pmp,
            tc.tile_pool(name="po", bufs=2, space="PSUM") as pop,
        ):
            # ---- persistent loads -------------------------------------
            # idx tiles FIRST: every gather waits on them, and HWDGE DMAs
            # execute in FIFO order per issuing engine
            idx0_sb = constp.tile([128, ec // 16], I16)
            nc.sync.dma_start(idx0_sb[:], idx0_d[:])
            idx1_sb = constp.tile([128, ec // 16], I16)
            nc.sync.dma_start(idx1_sb[:], idx1_d[:])

            # whole x table in one DMA (pre-transposed on the host: one
            # contiguous 20KB descriptor per partition)
            xall = constp.tile([128, NPAIR * 2, XW], FP8)
            nc.sync.dma_start(xall[:], xhl_d[:])
            xhl_t = [xall[:, i * 2 * prs:(i + 1) * 2 * prs, :]
                     for i in range(n_xs)]

            wpack = constp.tile([128, 5 * 2 * H + 2], BF16)
            nc.sync.dma_start(wpack[:], wpack_d[:])
            w_sb = {nm: wpack[:, i * 2 * H:(i + 1) * 2 * H]
                    .rearrange("p (k h) -> p k h", k=2)
                    for i, nm in enumerate(("wat", "wbt", "w1t", "w2t", "w3t"))}
            wlt_sb = wpack[:, 10 * H:10 * H + 2].rearrange(
                "p (k o) -> p k o", k=2)
            bpack = constp.tile([128, 11], FP32)
            nc.sync.dma_start(bpack[:], bpack_d[:])
            b_sb = {nm: bpack[:, i * 2:(i + 1) * 2]
                    .rearrange("p (t o) -> p t o", t=2)
                    for i, nm in enumerate(("ba", "bb", "b1", "b2", "b3"))}
            beta_sb = bpack[:, 10:11]
            bl_sb = constp.tile([1, 1], FP32)
            nc.sync.dma_start(bl_sb[:], bl_d[:])

            # hoist every gather: descriptor generation serializes on the
            # gpsimd engine, so issue all eight back-to-back up front
            gt = {}
            for s in range(n_eh):
                isl = slice(s * (ehw // 16), (s + 1) * (ehw // 16))
                for nm, isb in (("g0", idx0_sb), ("g1", idx1_sb)):
                    g = gathp.tile([128, PB // 128, ehw], FP8,
                                   tag=f"{nm}_{s}")
                    nc.gpsimd.dma_gather(g[:], adjp_d[:], isb[:, isl],
                                         ehw, ehw, elem_size=PB,
                                         transpose=True)
                    gt[nm, s] = g
            ghl_t = {}
            for s in range(n_eh):
                isl = slice(s * (ehw // 16), (s + 1) * (ehw // 16))
                for nm, isb in (("xi", idx0_sb), ("xj", idx1_sb)):
                    ghl = ghlp.tile([128, 4, ehw], BF16, tag=f"ghl{nm}{s}")
                    nc.gpsimd.dma_gather(ghl[:], xg_d[:], isb[:, isl],
                                         ehw, ehw, elem_size=2 * D,
                                         transpose=True)
                    ghl_t[nm, s] = ghl

            xcnT = constp.tile([128, 2, ec], BF16)   # feature-major xcn
            out_sb = constp.tile([1, ec], FP32)

            # ---- MLP for one edge-half (ehw edges), feature-major f32r;
            # emitted mid-loop so its PE work overlaps gather-bound spans.
            def lin_h(src_ap, wname, bname, relu, dst):
                w, bias = w_sb[wname], b_sb[bname]
                for t in range(2):
                    pm = pmp.tile([128, ehw], FP32, tag="pm")
                    for k in range(2):
                        nc.tensor.matmul(
                            pm[:], w[:, k, t * 128:(t + 1) * 128],
                            src_ap[:, k, :], start=(k == 0), stop=(k == 1))
                    dsl = dst[:, t, :]
                    if t % 2 == 0:
                        nc.scalar.activation(
                            dsl, pm[:], AF.Relu if relu else AF.Identity,
                            bias=bias[:, t, :])
                    elif relu:
                        nc.vector.tensor_scalar(
                            dsl, pm[:], bias[:, t, :], 0.0, ALU.add, ALU.max)
                    else:
                        nc.vector.tensor_scalar_add(dsl, pm[:], bias[:, t, :])
                return dst

            def mlp_half(hh):
                esl = slice(hh * ehw, (hh + 1) * ehw)
                xiT = actp.tile([128, 2, ehw], FP32, tag="act")
                xjT = actp.tile([128, 2, ehw], FP32, tag="act")
                for dst, nm in ((xiT, "xi"), (xjT, "xj")):
                    ghl = ghl_t[nm, hh]
                    nc.vector.tensor_add(dst[:], ghl[:, 0:2, :], ghl[:, 2:4, :])
                pT = actp.tile([128, 2, ehw], BF16, tag="actb")
                nc.vector.tensor_mul(pT[:], xiT[:], xjT[:])
                u = lin_h(pT[:], "wat", "ba", True,
                          actp.tile([128, 2, ehw], BF16, tag="actb",
                                    name=f"u{hh}"))
                xijT = lin_h(u[:], "wbt", "bb", False,
                             actp.tile([128, 2, ehw], BF16, tag="actb",
                                       name=f"xijT{hh}"))
                h = xcnT[:, :, esl]
                for li, (wn, bn, rl) in enumerate((
                        ("w1t", "b1", True), ("w2t", "b2", True),
                        ("w3t", "b3", False))):
                    h = lin_h(h, wn, bn, rl,
                              actp.tile([128, 2, ehw], BF16, tag="actb",
                                        name=f"h{hh}_{li}"))[:]
                nc.vector.tensor_scalar_mul(h, h, beta_sb[:])
                nc.vector.tensor_add(h, h, xijT[:])
                po = pop.tile([1, ehw], FP32, tag="po")
                for k in range(2):
                    nc.tensor.matmul(po[:], wlt_sb[:, k, :], h[:, k, :],
                                     start=(k == 0), stop=(k == 1))
                nc.scalar.activation(out_sb[:, hh * ehw:(hh + 1) * ehw],
                                     po[:], AF.Identity, bias=bl_sb[:])

            # ---- main loop: packed gather -> AND -> planes -> xcn^T ---
            for sb in range(n_eh):
                g0, g1 = gt["g0", sb], gt["g1", sb]
                # cn = g0 AND g1 on int16 lanes (two packed bytes per lane)
                v0 = g0[:].rearrange("p a b -> p (a b)").bitcast(I16)
                v1 = g1[:].rearrange("p a b -> p (a b)").bitcast(I16)
                usz = PB * ehw // 256
                nc.vector.tensor_tensor(v0, v0, v1, ALU.bitwise_and)
                # bit-plane extraction: plane k = (v >> (k-4)) & 0x1010,
                # one fused shift+mask op each; surviving bit = fp8 2^-5
                pl = planep.tile([128, 8, C2, ehw, 2], FP8, tag="pl")
                pli = pl[:].rearrange("p k c e t -> p (k c e t)").bitcast(I16)
                for k in range(8):
                    dstk = pli[:, k * usz:(k + 1) * usz]
                    if k < 4:
                        nc.vector.tensor_scalar(
                            dstk, v0, 4 - k, 0x1010,
                            ALU.logical_shift_left, ALU.bitwise_and)
                    elif k == 4:
                        nc.vector.tensor_scalar(
                            dstk, v0, 0x1010, 0, ALU.bitwise_and, ALU.bypass)
                    else:
                        nc.vector.tensor_scalar(
                            dstk, v0, k - 4, 0x1010,
                            ALU.logical_shift_right, ALU.bitwise_and)
                # 160 DoubleRow matmuls, x pairs stationary, planes moving:
                # px[c][d, e] += sum_par x[n, 128c+d] * cn[n, e]
                px = pxp.tile([128, NCH, ehw], FP32, tag="px")
                for g in range(NPAIR):
                    k, c2 = g // C2, g % C2
                    rhs = pl[:, k, c2].rearrange("p e t -> p t e")
                    xp = xhl_t[g // prs]
                    lg = g % prs
                    for c in range(NCH):
                        nc.tensor.matmul(
                            px[:, c, :],
                            xp[:, 2 * lg:2 * lg + 2, 128 * c:128 * (c + 1)],
                            rhs, start=(g == 0), stop=(g == NPAIR - 1),
                            perf_mode=DR)
                # xcn^T to bf16 (adding the lo half first when NHALF == 2)
                for t in range(2):
                    dstT = xcnT[:, t, sb * ehw:(sb + 1) * ehw]
                    if NHALF == 2:
                        xcn_sb = workp.tile([128, ehw], FP32, tag="xcn")
                        nc.scalar.activation(xcn_sb[:], px[:, t, :], AF.Copy)
                        nc.vector.tensor_add(dstT, px[:, t + 2, :], xcn_sb[:])
                    else:
                        nc.scalar.activation(dstT, px[:, t, :], AF.Copy)
                mlp_half(sb)

            nc.sync.dma_start(out_d[:], out_sb[:])

    nc.compile()
    return nc


def _wrap_idx(e_slice, ec):
    """Pack indices for dma_gather: [128, ec//16] int16, idx i at
    [i % 16, i // 16], replicated over the 8 groups of 16 partitions."""
    a = np.asarray(e_slice).astype(np.int16)
    w = a.reshape(ec // 16, 16).T.copy()
    return np.ascontiguousarray(np.tile(w, (8, 1)))


def _plane_row(npad=NPAD):
    """row_of_node[n]: x-table row for node n under the plane-major layout.
    Packed byte m = 256*c2 + 2*r + par of a gathered row lands at partition
    r, and bit k of that byte goes to plane k; the DoubleRow pair for
    (k, c2) contracts par=0,1, so node n = 8*m + k must live at table row
    ((k*C2 + c2)*2 + par)*128 + r."""
    n = np.arange(npad)
    k, m = n % 8, n // 8
    c2, rem = m // 256, m % 256
    r, par = rem // 2, rem % 2
    return ((k * C2 + c2) * 2 + par) * 128 + r


def prepare_inputs(x, adj, edge, W1, b1, W2, b2, W3, b3, Wa, ba, Wb, bb,
                   Wl, bl, beta, n=N, npad=NPAD, ncores=N_CORES):
    x = np.asarray(x, np.float32)
    adj = np.asarray(adj)
    edge = np.asarray(edge)
    ec = edge.shape[0] // ncores

    # bitpacked adjacency, little bit order: byte m bit k = adj[:, 8m+k]
    adjp = np.packbits(adj.astype(bool), axis=1, bitorder="little")
    if adjp.shape[1] < PB:
        adjp = np.pad(adjp, ((0, 0), (0, PB - adjp.shape[1])))
    adjp = np.ascontiguousarray(adjp).view(FP8_NP)

    # fp8 split of 32*x ([hi | lo] when NHALF == 2), rows in plane-major
    # order, then pre-transposed to [partition, group, col] for the load
    xs = XSCALE * x
    x_hi = xs.astype(FP8_NP)
    xhl = np.zeros((npad, XW), FP8_NP)
    row = _plane_row(npad)
    xhl[row[:n], :D] = x_hi
    if NHALF == 2:
        xhl[row[:n], D:] = (xs - x_hi.astype(np.float32)).astype(FP8_NP)
    xhl = np.ascontiguousarray(
        xhl.reshape(npad // 128, 128, XW).transpose(1, 0, 2))

    # bf16 hi/lo split of x, natural row order, for the xi/xj gathers
    xg_hi = x.astype(ml_dtypes.bfloat16)
    xg_lo = (x - xg_hi.astype(np.float32)).astype(ml_dtypes.bfloat16)
    xg = np.zeros((npad, 2 * D), ml_dtypes.bfloat16)
    xg[:n, :D] = xg_hi
    xg[:n, D:] = xg_lo

    # wpack[p, i*512:(i+1)*512] = W_i.T[(k p)] -> [p, (k h)]; +wlt at the end
    BF = ml_dtypes.bfloat16
    wpack = np.zeros((128, 5 * 2 * H + 2), BF)
    for i, W in enumerate((Wa, Wb, W1, W2, W3)):
        wt = np.asarray(W, np.float32).T.astype(BF)      # [D, H], f = 128k + p
        wpack[:, i * 2 * H:(i + 1) * 2 * H] = \
            wt.reshape(2, 128, H).transpose(1, 0, 2).reshape(128, 2 * H)
    wlt = np.asarray(Wl, np.float32).T.astype(BF)        # [H, 1]
    wpack[:, 10 * H:10 * H + 2] = wlt.reshape(2, 128).T
    bpack = np.zeros((128, 11), np.float32)
    for i, b in enumerate((ba, bb, b1, b2, b3)):
        bpack[:, i * 2:(i + 1) * 2] = np.asarray(b, np.float32).reshape(2, 128).T
    bpack[:, 10] = np.asarray(beta, np.float32).reshape(-1)[0]

    common = dict(
        adjp=adjp, xhl=xhl, xg=xg, wpack=wpack, bpack=bpack,
        bl=np.asarray(bl, np.float32).reshape(1, 1),
    )
    in_maps = []
    for c in range(ncores):
        sl = slice(c * ec, (c + 1) * ec)
        m = dict(common)
        m["idx0"] = _wrap_idx(edge[sl, 0], ec)
        m["idx1"] = _wrap_idx(edge[sl, 1], ec)
        in_maps.append(m)
    return in_maps


_CACHE = {}


def _get_program():
    if "nc" not in _CACHE:
        _CACHE["nc"] = build_program()
    return _CACHE["nc"]


def run(in_maps, **kw):
    nc = _get_program()
    return run_bass_kernel_spmd(nc, in_maps, list(range(N_CORES)), **kw)


def kernel(**inputs):
    in_maps = prepare_inputs(**inputs)
    res = run(in_maps)
    out = np.concatenate([res.results[c]["out"][0] for c in range(N_CORES)])
    return out.reshape(E, 1).astype(np.float32)
